# revision 40
# baseline (speedup 1.0000x reference)
"""Sparse neighbor-attention (point transformer style) on 8 Trainium2 cores.

Strategy (segment/data parallel):
- Points sharded contiguously: core c owns points [c*6250, (c+1)*6250).
- Every core computes the full k|v table [50048, 512] bf16 in its own HBM.
  The SWDGE dma_gather's int16 indices are signed, so the gather base is
  placed INSIDE the table at row TBASE=17280: encoded idx v = r - TBASE
  spans [-17280, 32767] and every row is reachable (negative v gathers
  below the base via the ucode's signed offset multiply).
- Per 128-point tile: 2 dma_gathers (slots 0-7, 8-15); scores on DVE via a
  bf16 2x add-tree; softmax on ACT/DVE; the weighted-v K-sum and transpose
  are fused into 32 accumulating PE transposes; projection on PE.
- The final entry of each gather's index stream must be non-negative (the
  Q7 ucode trims trailing negatives), so each point's two largest neighbor
  ids sit at slots 7 and 15 (softmax+sum are slot-permutation invariant).

Self-contained: builds the Bass program, shards inputs on the host, runs via
run_bass_kernel_spmd on cores 0-7, reassembles the full [50000, 256] output.
"""
import math
import os
import sys
from contextlib import ExitStack

import numpy as np

for _p in ('/opt/trn_rl_repo', '/root/.axon_site/_ro/trn_rl_repo'):
    if os.path.isdir(_p) and _p not in sys.path:
        sys.path.append(_p)

import ml_dtypes
import concourse.bass as bass
import concourse.mybir as mybir
import concourse.tile as tile
from concourse.masks import make_identity
from concourse.bass_utils import run_bass_kernel_spmd
from concourse import library_config
from concourse.library_overlay import lower_extended_insts

# ---------------------------------------------------------------------------
# Workaround: this container's walrus rejects >2 sync waits on one
# instruction ("Too many sync wait commands" in setupSyncWait). Split excess
# waits onto same-engine nops committed immediately before the instruction.
_MAX_WAITS = 1
_orig_commit = tile.TileContext._commit_instruction


def _commit_split_waits(self, inst, lazy_reg_writes=True):
    si = getattr(inst, "sync_info", None)
    if si is not None and len(si.on_wait) > _MAX_WAITS:
        waits = list(si.on_wait)
        keep = waits[:_MAX_WAITS]
        rest = waits[_MAX_WAITS:]
        si.on_wait.clear()
        for w in keep:
            si.on_wait.append(w)
        for i in range(0, len(rest), _MAX_WAITS):
            nop = mybir.InstNoOp(
                name=self.nc.get_next_instruction_name(),
                engine=inst.engine,
                bass_nofuse=True,
                sync_info=mybir.SyncInfo(
                    on_wait=rest[i:i + _MAX_WAITS], on_update=[]),
            )
            _orig_commit(self, nop, lazy_reg_writes=False)
    return _orig_commit(self, inst, lazy_reg_writes=lazy_reg_writes)


tile.TileContext._commit_instruction = _commit_split_waits


def _drain_and_barrier_split(self, tick_clock, wait_clock):
    import bass_rust as _br
    carrier = self.nc.sync.nop(nofuse=True, hint="drain_wait_carrier")
    wait_clock.add_sem_waits(carrier.ins,
                             _br.ScopedClock({None: tick_clock.global_clock}))
    si = carrier.ins.sync_info
    waits = list(si.on_wait) if si is not None else []
    if si is not None:
        si.on_wait.clear()
    for w in waits:
        nop = self.nc.sync.nop(nofuse=True, hint="drain_wait_split")
        nsi = nop.ins.sync_info
        if nsi is None:
            nop.ins.sync_info = mybir.SyncInfo(on_wait=[w], on_update=[])
        else:
            nsi.on_wait.append(w)
    self.nc.sync.drain()
    self.nc.all_engine_barrier()
    assert self.sems is not None
    popped = self.nc._tile_sem_poison_stack.pop()
    assert popped is self._sem_poison
    self.nc.clear_and_free_semaphores(list(self.sems.allocated().values()))
    self.nc.all_engine_barrier()


tile.TileContext._drain_and_barrier = _drain_and_barrier_split
# ---------------------------------------------------------------------------

P = 128
F32 = mybir.dt.float32
BF16 = mybir.dt.bfloat16
I32 = mybir.dt.int32
I16 = mybir.dt.int16
ALU = mybir.AluOpType
AXT = mybir.AxisListType
ACTF = mybir.ActivationFunctionType

N_CORES = 8
N_TOTAL = 50000
K = 16
DIM = 256
H = 8
HD = DIM // H
D2 = 2 * DIM

NPAD = 50048             # TA * P
TA = NPAD // P           # 391
# Gather base sits INSIDE the table at row TBASE: encoded idx v = r - TBASE
# spans [-17280, 32767], all within signed int16, so every row is reachable
# with no duplicated segment.
TBASE = NPAD - 32768     # 17280
TROWS = NPAD

SCRATCH = 36864          # SWDGE ring: 2304 descriptors

LAST_EXEC_NS = None
_PROGRAM_CACHE = {}


def _bcast_ap(ap, insert_axis, count):
    dims = list(ap.ap)
    dims.insert(insert_axis, [0, count])
    return bass.AP(ap.tensor, ap.offset, dims)


def _build(n_total, n_own):
    TO = math.ceil(n_own / P)
    NOWN_PAD = TO * P

    nc = bass.Bass(dynamic_dma_scratch_size=SCRATCH)
    featsT = nc.dram_tensor("featsT", [DIM, NPAD], BF16, kind="ExternalInput")
    fownT = nc.dram_tensor("fownT", [DIM, NOWN_PAD], BF16, kind="ExternalInput")
    wkvT = nc.dram_tensor("wkvT", [DIM, D2], BF16, kind="ExternalInput")
    wqT = nc.dram_tensor("wqT", [DIM, DIM], BF16, kind="ExternalInput")
    bq = nc.dram_tensor("bq", [1, DIM], BF16, kind="ExternalInput")
    wpT = nc.dram_tensor("wpT", [DIM, DIM], BF16, kind="ExternalInput")
    bp = nc.dram_tensor("bp", [1, DIM], BF16, kind="ExternalInput")
    # per tile: 2 gathers x 64 cols of wrapped u16-as-i16 ids
    idx = nc.dram_tensor("idx", [P, TO * 128], I16, kind="ExternalInput")
    kv = nc.dram_tensor("kv", [TROWS, D2], BF16, kind="Internal")
    out = nc.dram_tensor("out", [NOWN_PAD, DIM], F32, kind="ExternalOutput")

    with tile.TileContext(nc) as tc, ExitStack() as ctx:
        singles = ctx.enter_context(tc.tile_pool(name="singles", bufs=1))
        fpool = ctx.enter_context(tc.tile_pool(name="fpool", bufs=2))
        kpool = ctx.enter_context(tc.tile_pool(name="kpool", bufs=2))
        gpool = ctx.enter_context(tc.tile_pool(name="gpool", bufs=2))
        cpool = ctx.enter_context(tc.tile_pool(name="cpool", bufs=2))
        cpool3 = ctx.enter_context(tc.tile_pool(name="cpool3", bufs=2))
        opool = ctx.enter_context(tc.tile_pool(name="opool", bufs=2))
        psumA = ctx.enter_context(tc.tile_pool(name="psumA", bufs=2, space="PSUM"))
        psumO = ctx.enter_context(tc.tile_pool(name="psumO", bufs=1, space="PSUM"))
        psumT = ctx.enter_context(tc.tile_pool(name="psumT", bufs=3, space="PSUM"))


        w_kv = singles.tile([P, 2, D2], BF16)
        nc.sync.dma_start(out=w_kv[:], in_=wkvT[:, :].rearrange("(b p) m -> p b m", p=P))
        w_q = singles.tile([P, 2, DIM], BF16)
        nc.sync.dma_start(out=w_q[:], in_=wqT[:, :].rearrange("(b p) m -> p b m", p=P))
        w_p = singles.tile([P, 2, DIM], BF16)
        nc.sync.dma_start(out=w_p[:], in_=wpT[:, :].rearrange("(b p) m -> p b m", p=P))
        b_q = singles.tile([1, DIM], BF16)
        nc.sync.dma_start(out=b_q[:], in_=bq[:, :])
        b_p = singles.tile([1, DIM], BF16)
        nc.sync.dma_start(out=b_p[:], in_=bp[:, :])
        idx_all = singles.tile([P, TO * 128], I16)
        nc.sync.dma_start(out=idx_all[:], in_=idx[:, :])
        ones = singles.tile([1, P], BF16)
        nc.vector.memset(ones[:], 1.0)
        ident = singles.tile([P, P], BF16)
        make_identity(nc, ident[:])
        q_all = singles.tile([P, TO, DIM], BF16)
        # library switch AFTER the last standard Pool op (iota/affine_select):
        # under mlp (index 3) Pool may only run the custom DMA instructions
        nc.gpsimd.load_library(library_config.mlp)
        nreg = nc.gpsimd.to_reg(1024)

        # ---- phase A: k|v table (main + duplicated hi segment) -----------
        # k bias cancels in softmax; v bias folds into proj bias (host).
        # 8 point-tiles per iteration: one big feats load, four matmul+copy
        # rounds into a staging tile, one (or two) big stores.
        GB = 8
        for st in range(math.ceil(TA / GB)):
            t0s = GB * st
            nt = min(GB, TA - t0s)
            ft = fpool.tile([P, 2, GB * P], BF16, tag="ft")
            nc.sync.dma_start(
                out=ft[:, :, 0:nt * P],
                in_=featsT[:, t0s * P:(t0s + nt) * P]
                .rearrange("(b p) i -> p b i", p=P))
            kvsb = kpool.tile([P, GB, D2], BF16, tag="kvsb")
            for r in range(math.ceil(nt / 2)):
                u0 = 2 * r
                nu = min(2, nt - u0)
                kvps = psumA.tile([P, 2, D2], F32, tag="kvps")
                for u in range(nu):
                    nc.tensor.matmul(
                        out=kvps[:, u, :],
                        lhsT=ft[:, 0, (u0 + u) * P:(u0 + u + 1) * P],
                        rhs=w_kv[:, 0, :], start=True, stop=False)
                    nc.tensor.matmul(
                        out=kvps[:, u, :],
                        lhsT=ft[:, 1, (u0 + u) * P:(u0 + u + 1) * P],
                        rhs=w_kv[:, 1, :], start=False, stop=True)
                if r % 2 == 0:
                    nc.scalar.copy(out=kvsb[:, u0:u0 + nu, :],
                                   in_=kvps[:, 0:nu, :])
                else:
                    nc.vector.tensor_copy(out=kvsb[:, u0:u0 + nu, :],
                                          in_=kvps[:, 0:nu, :])
            row0 = t0s * P
            h1 = min(4, nt)
            nc.scalar.dma_start(
                out=kv[row0:row0 + h1 * P, :]
                .rearrange("(u p) m -> p u m", p=P),
                in_=kvsb[:, 0:h1, :])
            if nt > 4:
                nc.scalar.dma_start(
                    out=kv[row0 + 4 * P:row0 + nt * P, :]
                    .rearrange("(u p) m -> p u m", p=P),
                    in_=kvsb[:, 4:nt, :])

        # ---- phase B: q for own points -----------------------------------
        for tb in range(TO):
            fo = fpool.tile([P, 2, P], BF16, tag="fo")
            nc.sync.dma_start(
                out=fo[:],
                in_=fownT[:, tb * P:(tb + 1) * P].rearrange("(b p) i -> p b i", p=P))
            qps = psumO.tile([P, DIM], F32, tag="ops")
            nc.tensor.matmul(out=qps[:], lhsT=fo[:, 0, :], rhs=w_q[:, 0, :],
                             start=True, stop=False)
            nc.tensor.matmul(out=qps[:], lhsT=fo[:, 1, :], rhs=w_q[:, 1, :],
                             start=False, stop=False)
            nc.tensor.matmul(out=qps[:], lhsT=ones[:1, :], rhs=b_q[:1, :],
                             start=False, stop=True)
            nc.scalar.copy(out=q_all[:, tb, :], in_=qps[:])

        # ---- phase C: attention + projection -----------------------------
        for t in range(TO):
            kvg = gpool.tile([P, K, D2], BF16, tag="kvg", bufs=4)
            for s in range(2):
                nc.gpsimd.dma_gather(
                    out_ap=kvg[:, s * 8:(s + 1) * 8, :],
                    in_ap=kv[TBASE:, :],
                    idxs_ap=idx_all[:, t * 128 + s * 64:t * 128 + (s + 1) * 64],
                    num_idxs=1024, num_idxs_reg=nreg, elem_size=D2)
            # scores: q . k per (point, slot, head); d-fold tree in bf16 2x
            pv = kvg[:, :, 0:DIM].rearrange("p k (h x) -> p k h x", h=H)
            r1 = cpool.tile([P, K, H, 16], BF16, tag="r1", bufs=3)
            r2 = cpool.tile([P, K, H, 8], BF16, tag="r2", bufs=3)
            r3 = cpool.tile([P, K, H, 4], BF16, tag="r3", bufs=3)
            scores = cpool.tile([P, K * H], F32, tag="scores", bufs=3)
            sv = scores[:].rearrange("p (k h) -> p k h", h=H)
            for hh in range(2):
                ks = slice(hh * 8, (hh + 1) * 8)
                nc.vector.tensor_tensor(
                    out=kvg[:, ks, 0:DIM], in0=kvg[:, ks, 0:DIM],
                    in1=_bcast_ap(q_all[:, t, :], 1, 8), op=ALU.mult)
                nc.vector.tensor_tensor(out=r1[:, ks], in0=pv[:, ks, :, 0:16],
                                        in1=pv[:, ks, :, 16:32], op=ALU.add)
                nc.vector.tensor_tensor(out=r2[:, ks], in0=r1[:, ks, :, 0:8],
                                        in1=r1[:, ks, :, 8:16], op=ALU.add)
                nc.vector.tensor_tensor(out=r3[:, ks], in0=r2[:, ks, :, 0:4],
                                        in1=r2[:, ks, :, 4:8], op=ALU.add)
                nc.vector.tensor_reduce(out=sv[:, ks], in_=r3[:, ks],
                                        axis=AXT.X, op=ALU.add)
            # softmax (no max-subtraction: scores are O(+-8), exp fp32-safe)
            ex = cpool.tile([P, K * H], BF16, tag="ex", bufs=3)
            nc.scalar.activation(out=ex[:], in_=scores[:], func=ACTF.Exp)
            den = cpool.tile([P, H], F32, tag="den")
            nc.vector.tensor_reduce(
                out=den[:], in_=ex[:].rearrange("p (k h) -> p h k", h=H),
                axis=AXT.X, op=ALU.add)
            rec = cpool.tile([P, H], F32, tag="rec")
            nc.vector.reciprocal(rec[:], den[:])
            attn = cpool.tile([P, K, H], BF16, tag="attn", bufs=3)
            nc.vector.tensor_tensor(
                out=attn[:], in0=ex[:].rearrange("p (k h) -> p k h", h=H),
                in1=_bcast_ap(rec[:], 1, K), op=ALU.mult)
            # expand normalized weights over head-dim on ACT; process the K
            # axis in halves so ACT/DVE/PE overlap within the tile
            aexp = cpool3.tile([P, K, DIM], BF16, tag="aexp", bufs=3)
            xps = psumT.tile([P, 2, P], F32, tag="xps")
            for hh in range(2):
                ks = slice(hh * 8, (hh + 1) * 8)
                nc.scalar.copy(
                    out=aexp[:, ks, :].rearrange("p k (h d) -> p k h d", h=H),
                    in_=_bcast_ap(attn[:, ks, :], 3, HD))
                nc.vector.tensor_tensor(out=kvg[:, ks, DIM:D2],
                                        in0=kvg[:, ks, DIM:D2],
                                        in1=aexp[:, ks, :], op=ALU.mult)
            # one accumulation group at a time (the two b-halves share a
            # PSUM bank, and a bank allows one open group)
            for b in range(2):
                for j in range(K):
                    nc.tensor.matmul(out=xps[:, b, :],
                                     lhsT=kvg[:, j, DIM + b * P:DIM + (b + 1) * P],
                                     rhs=ident[:],
                                     start=(j == 0), stop=(j == K - 1))
            xT = opool.tile([P, 2, P], BF16, tag="xT")
            nc.scalar.copy(out=xT[:], in_=xps[:])
            pps = psumO.tile([P, DIM], F32, tag="ops")
            nc.tensor.matmul(out=pps[:], lhsT=xT[:, 0, :], rhs=w_p[:, 0, :],
                             start=True, stop=False)
            nc.tensor.matmul(out=pps[:], lhsT=xT[:, 1, :], rhs=w_p[:, 1, :],
                             start=False, stop=False)
            nc.tensor.matmul(out=pps[:], lhsT=ones[:1, :], rhs=b_p[:1, :],
                             start=False, stop=True)
            osb = opool.tile([P, DIM], F32, tag="osb")
            nc.scalar.copy(out=osb[:], in_=pps[:])
            nc.sync.dma_start(out=out[t * P:(t + 1) * P, :], in_=osb[:])

    nc.finalize()
    lower_extended_insts(nc)
    return nc


def _host_prep(feats, index_1, qkv_w, qkv_b, proj_w, proj_b):
    bf16 = ml_dtypes.bfloat16
    N = feats.shape[0]
    scale = HD ** -0.5
    n_own = N // N_CORES
    TO = math.ceil(n_own / P)
    NOWN_PAD = TO * P

    featsT = np.zeros((DIM, NPAD), dtype=bf16)
    featsT[:, :N] = np.asarray(feats, dtype=np.float32).T.astype(bf16)
    qkv_w = np.asarray(qkv_w, dtype=np.float32)
    qkv_b = np.asarray(qkv_b, dtype=np.float32)
    wqT = np.ascontiguousarray((qkv_w[0:DIM] * scale).astype(bf16).T)
    bqv = (qkv_b[0:DIM] * scale).astype(bf16).reshape(1, -1)
    wkvT = np.ascontiguousarray(qkv_w[DIM:3 * DIM].astype(bf16).T)
    proj_w = np.asarray(proj_w, np.float32)
    wpT = np.ascontiguousarray(proj_w.astype(bf16).T)
    bv = qkv_b[2 * DIM:3 * DIM]
    bpv = (np.asarray(proj_b, np.float32) + proj_w @ bv).astype(bf16).reshape(1, -1)

    nbr = np.asarray(index_1).reshape(N, K).astype(np.int64)
    # per-point slot permutation: two LARGEST ids at slots 7 and 15 so the
    # final entry of each 1024-id gather stream encodes non-negative
    # (v = r - TBASE >= 0 iff r >= TBASE)
    srt = np.sort(nbr, axis=1)
    perm = np.concatenate([srt[:, 0:7], srt[:, 14:15], srt[:, 7:14],
                           srt[:, 15:16]], axis=1)
    perm = (perm - TBASE).astype(np.int64)

    in_maps = []
    for c in range(N_CORES):
        c0 = c * n_own
        fown = featsT[:, c0:c0 + NOWN_PAD]
        if fown.shape[1] < NOWN_PAD:
            fown = np.concatenate(
                [fown, np.zeros((DIM, NOWN_PAD - fown.shape[1]), dtype=bf16)],
                axis=1)
        fown = np.ascontiguousarray(fown)
        nb = np.zeros((NOWN_PAD, K), dtype=np.int64)
        end = min(c0 + NOWN_PAD, N)
        nb[:end - c0] = perm[c0:end]
        # stream for tile t, gather s: positions i -> (p=i%128, slot=s*8+i//128)
        blocks = nb.reshape(TO, P, 2, 8)          # [t, p, s, k_local]
        streams = blocks.transpose(0, 2, 3, 1)    # [t, s, k_local, p]
        streams = streams.reshape(TO, 2, 1024)
        # check tail protection: last entry of each stream must encode >= 0
        assert (streams[:, :, -1] >= 0).all(), "tail protection violated"
        # wrap: position i -> partition i%16, col i//16; replicate to 128
        w = streams.reshape(TO, 2, 64, 16).transpose(0, 1, 3, 2)  # [t,s,16,64]
        idx_host = np.zeros((TO, P, 128), dtype=np.int16)
        for s in range(2):
            blk = w[:, s]                          # [TO, 16, 64]
            idx_host[:, :, s * 64:(s + 1) * 64] = np.tile(blk, (1, 8, 1))
        idx_host = np.ascontiguousarray(
            idx_host.transpose(1, 0, 2).reshape(P, TO * 128))
        in_maps.append({
            "featsT": featsT, "fownT": fown,
            "wkvT": wkvT, "wqT": wqT, "bq": bqv,
            "wpT": wpT, "bp": bpv, "idx": idx_host,
        })
    return in_maps, n_own


def kernel(feats, xyz, index_0, index_1, index_0_offsets, n_max,
           qkv_w, qkv_b, proj_w, proj_b, _trace=False):
    global LAST_EXEC_NS
    N = feats.shape[0]
    n_own = N // N_CORES

    key = (N, n_own)
    if key not in _PROGRAM_CACHE:
        _PROGRAM_CACHE[key] = _build(N, n_own)
    nc = _PROGRAM_CACHE[key]

    in_maps, n_own = _host_prep(feats, index_1, qkv_w, qkv_b, proj_w, proj_b)
    try:
        res = run_bass_kernel_spmd(nc, in_maps, core_ids=list(range(N_CORES)),
                                   trace=_trace)
    except Exception:
        if not _trace:
            raise
        res = run_bass_kernel_spmd(nc, in_maps, core_ids=list(range(N_CORES)),
                                   trace=False)
    LAST_EXEC_NS = res.exec_time_ns
    outs = [np.asarray(res.results[c]["out"])[:n_own] for c in range(N_CORES)]
    return np.concatenate(outs, axis=0).astype(np.float32)
